# revision 33
# baseline (speedup 1.0000x reference)
"""GCN layer (2-hop SpMM + per-hop Linear/ReLU) on 8 Trainium2 NeuronCores.

Strategy (dst-sharded pull, both hops; single AllGather between them):
  - Nodes sharded 1250/core; each core owns the edges pointing at its
    shard. Both hops use the SAME edge set, so one host-built scatter
    table S serves both: per dst block, edges are grouped by UNIQUE src
    (dedup: each distinct src row is gathered once per block) into
    128-lane chunks; S[lane, chunk, m] = sum of edge weights from that
    src to local dst m.
  - The SWDGE descriptor-generation rate on GpSimd Q7 cores 0/1 (~4.7ns
    per gathered row, 2-way overlap max) is the kernel's critical
    resource; ncfw collectives also occupy the GpSimd queue for their
    full duration. The layout therefore minimizes that serial chain:
    hop-1 gathers (local replicated h0) -> one AllGather of the 1.25MB
    h1 shard -> hop-2 gathers (local h1_blocked copy).
  - Per hop/block: dma_gather pulls rows (bf16) into SBUF G tiles;
    TensorE computes psum += S.T @ G per chunk (scatter-add; D_norm is
    folded into S host-side, valid for both hops since they share dst
    sharding); VectorE evicts psum to bf16. The feat-major copy for the
    linear layers is built with TensorE transpose-mode matmuls + DVE
    copies — NOT dma_start_transpose: Tile serializes DMA-transposes
    against SWDGE gathers (HW-deadlock workaround), which was measured
    to cap the gather pipeline at ~2 blocks in flight.
  - Linear stage runs feat-major: outT[fo, n] = relu(W.T @ hT + b), bias
    and relu fused in one ScalarE activation. linear(0) warms TensorE at
    the start; linear(1) is interleaved into early hop-2 blocks.
  - HWDGE traffic is split across both rings: nc.sync (SP) carries the
    urgent const loads + transposes, nc.scalar (ACT) carries the hop-2
    index table, h1 shard write-out and all output writes.
"""

import sys

sys.path.insert(0, "/opt/trn_rl_repo")

import numpy as np
import ml_dtypes

import concourse.bass as bass
import concourse.bacc as bacc
import concourse.mybir as mybir
import concourse.tile as tile
from concourse import library_config
from concourse.bass_utils import run_bass_kernel_spmd
from concourse.tile_rust import add_dep_helper

N_NODES = 10000
N_EDGES = 160000
D = 512
ORDER = 2
N_CORES = 8
SHARD = N_NODES // N_CORES          # 1250
BLKS = (SHARD + 127) // 128         # 10 dst blocks per core
BLK_SZ = [min(128, SHARD - b * 128) for b in range(BLKS)]  # [128]*9 + [98]
FI = D // 128                       # 4 feat-in chunks
FO = D // 128                       # 4 feat-out tiles
NGRPS = [512, 512, SHARD - 1024]    # node groups for linear stage
PAD_SHARD = BLKS * 128              # 1280 rows per core in blocked layout
BF16 = ml_dtypes.bfloat16


def _split_excess_waits(nc, max_waits=1):
    """This walrus build rejects >1 sync wait per instruction (and any on a
    Drain). Hoist excess SyncWaits onto InstNoOp carriers inserted just
    before, on the same engine — waits execute in program order, so
    semantics are preserved."""
    for fn in nc.m.functions:
        for bb in fn.blocks:
            new = []
            changed = False
            for inst in bb.instructions:
                si = inst.sync_info
                cap = 0 if isinstance(inst, mybir.InstDrain) else max_waits
                if si is not None and len(si.on_wait) > cap:
                    waits = list(si.on_wait)
                    excess = waits[:-cap] if cap else waits
                    keep = waits[-cap:] if cap else []
                    for g in range(0, len(excess), max_waits):
                        nop = mybir.InstNoOp(name=f"{inst.name}-ws{g}", ins=[], outs=[])
                        nop.engine = inst.engine
                        nop.sync_info = mybir.SyncInfo(
                            on_wait=excess[g:g + max_waits], on_update=[])
                        new.append(nop)
                    si.on_wait = keep
                    changed = True
                new.append(inst)
            if changed:
                bb.instructions = new


def _assign_gather_queues(nc):
    """Tile locks each DMASW sem lane to SWDGE queue lane%4; route every
    gather through the queue matching its (scheduler-assigned) sem lane so
    the 4 SWDGE queues actually run in parallel."""
    for fn in nc.m.functions:
        for bb in fn.blocks:
            for inst in bb.instructions:
                if isinstance(inst, mybir.InstDMAGatherAnt):
                    si = inst.sync_info
                    if not si:
                        continue
                    for u in si.on_update:
                        nm = getattr(u, "ant_name", "") or ""
                        if nm.startswith("DMASW"):
                            lane = int(nm[5:].split("_")[0])
                            inst.queue_num = lane % 4
                            break


def _blocked_row(s):
    """Row of global node s in the AllGather output layout
    h1_blocked[core * 1280 + local_row]."""
    return (s // SHARD) * PAD_SHARD + (s % SHARD)


def _preprocess(edge_w, src, dst, d_flat):
    """Host-side: shard edges by dst owner; per dst block, dedup by src
    into 128-lane chunks shared by both hops; D_norm[dst] is folded into
    the scatter weights. Returns per-core (nchk [BLKS], idx1, idx2, S)."""
    core_of = dst // SHARD
    per_core = []
    for i in range(N_CORES):
        sel = np.nonzero(core_of == i)[0]
        dl = dst[sel] - i * SHARD
        blk = dl // 128
        m = dl % 128
        s = src[sel]
        w = edge_w[sel] * d_flat[dst[sel]]
        nchk = np.zeros(BLKS, np.int64)
        cells = []
        for bi in range(BLKS):
            bm = blk == bi
            u, inv = np.unique(s[bm], return_inverse=True)
            nchk[bi] = max((len(u) + 127) // 128, 1)
            cells.append((u, inv, m[bm], w[bm]))
        ncht = int(nchk.sum())
        boff = np.concatenate([[0], np.cumsum(nchk)])
        idx1_f = np.zeros(ncht * 128, np.int16)
        s_t = np.zeros((128, ncht, 128), np.float32)
        for bi in range(BLKS):
            u, inv, mm, ww = cells[bi]
            lo = int(boff[bi])
            idx1_f[lo * 128: lo * 128 + len(u)] = u.astype(np.int16)
            np.add.at(s_t, (inv % 128, lo + inv // 128, mm), ww)
        per_core.append((nchk, idx1_f, s_t.astype(BF16)))
    return per_core


def _build_program(nchk, ncht):
    # The stock cost model prices SWDGE descriptor generation at 0.34ns
    # (calibrated on plain dma_start); dma_gather's data-dependent Q7 loop
    # measures ~8.7ns/index on HW. With the optimistic value the Tile list
    # scheduler interleaves linear-stage matmuls ahead of hop chains it
    # believes will stall, delaying the AllGather. Patch the constant so
    # schedule-time gather latencies match reality (no effect on the
    # generated instructions themselves, only their ordering).
    import concourse.hw_specs as hw_specs
    hw_specs.TRN2Spec.SWDGE_NS_PER_DESCRIPTOR = 7.0

    nc = bacc.Bacc("TRN2", num_swdge_queues=4)
    dt = mybir.dt

    h0_full = nc.declare_dram_parameter("h0_full", [N_NODES, D], dt.bfloat16, isOutput=False)
    h0t_shard = nc.declare_dram_parameter("h0t_shard", [128, FI, SHARD], dt.bfloat16, isOutput=False)
    idx1_in = nc.declare_dram_parameter("idx1", [128, ncht * 8], dt.int16, isOutput=False)
    s_in = nc.declare_dram_parameter("s", [128, ncht, 128], dt.bfloat16, isOutput=False)
    ident_in = nc.declare_dram_parameter("ident", [128, 128], dt.bfloat16, isOutput=False)
    w_in = nc.declare_dram_parameter("w", [128, ORDER + 1, FI, D], dt.bfloat16, isOutput=False)
    b_in = nc.declare_dram_parameter("bias", [128, ORDER + 1, FO], dt.float32, isOutput=False)
    out_t = nc.declare_dram_parameter("out_t", [(ORDER + 1) * D, SHARD], dt.float32, isOutput=True)

    # the AllGather of exact [SHARD]-row shards reproduces global node
    # order, so hop-2 gathers reuse idx1 (h1_blocked[src] = h1[src])
    h1_shard = nc.dram_tensor("h1_shard", [PAD_SHARD, D], dt.bfloat16)
    h1_blocked = nc.dram_tensor("h1_blocked", [N_NODES, D], dt.bfloat16,
                                addr_space="Shared")

    boff = np.concatenate([[0], np.cumsum(nchk)])

    with tile.TileContext(nc) as tc:
        nc.gpsimd.load_library(library_config.mlp)
        with (
            tc.tile_pool(name="const", bufs=1) as const,
            tc.tile_pool(name="gbuf", bufs=6) as gbuf,
            tc.tile_pool(name="evict", bufs=3) as evict,
            tc.tile_pool(name="lin", bufs=3) as lin,
            tc.tile_pool(name="psum", bufs=3, space=bass.MemorySpace.PSUM) as psum,
            tc.tile_pool(name="ptp", bufs=2, space=bass.MemorySpace.PSUM) as ptp,
            tc.tile_pool(name="psw", bufs=3, space=bass.MemorySpace.PSUM) as psw,
        ):
            # urgent first (sync ring): gather idx, weights, ht0 feed the
            # first gathers + linear(0); the big S table follows them.
            idx1_t = const.tile([128, ncht * 8], dt.int16)
            nc.sync.dma_start(idx1_t[:], idx1_in[:])
            ident_t = const.tile([128, 128], dt.bfloat16)
            nc.sync.dma_start(ident_t[:], ident_in[:])
            w_t = const.tile([128, ORDER + 1, FI, D], dt.bfloat16)
            nc.sync.dma_start(w_t[:], w_in[:])
            b_t = const.tile([128, ORDER + 1, FO], dt.float32)
            nc.sync.dma_start(b_t[:], b_in[:])
            ht = [const.tile([128, FI, PAD_SHARD], dt.bfloat16, tag=f"ht{k}",
                             name=f"ht{k}")
                  for k in range(ORDER + 1)]
            nc.sync.dma_start(ht[0][:, :, :SHARD], h0t_shard[:])
            # big table on the ACT ring so its load doesn't alias the
            # gathers' DMAHW wait lanes on the sync ring
            s_t = const.tile([128, ncht, 128], dt.bfloat16)
            nc.scalar.dma_start(s_t[:], s_in[:])

            def linear(k, fts=None, after=None):
                for ft in (range(FO) if fts is None else fts):
                    for gi, gsz in enumerate(NGRPS):
                        goff = sum(NGRPS[:gi])
                        pw = psw.tile([128, gsz], dt.float32, tag="pw", name="pw")
                        for fi in range(FI):
                            mm = nc.tensor.matmul(
                                pw[:], w_t[:, k, fi, ft * 128:(ft + 1) * 128],
                                ht[k][:, fi, goff:goff + gsz],
                                start=(fi == 0), stop=(fi == FI - 1))
                            if after is not None and fi == 0:
                                add_dep_helper(mm.ins, after.ins, sync=True,
                                               reason="linear after hop chain")
                        ob = lin.tile([128, gsz], dt.float32, tag="ob", name="ob")
                        nc.scalar.activation(
                            out=ob[:], in_=pw[:],
                            func=mybir.ActivationFunctionType.Relu,
                            bias=b_t[:, k, ft:ft + 1])
                        nc.scalar.dma_start(
                            out_t[k * D + ft * 128:k * D + (ft + 1) * 128,
                                  goff:goff + gsz], ob[:])

            def hop_block(bi, k, idx_t, src_ap):
                """One dst block of one SpMM hop (shared chunk table)."""
                nch = int(nchk[bi])
                off = int(boff[bi])
                g = gbuf.tile([128, nch, D], dt.bfloat16, tag="g", name="g")
                # fine-grained gather calls (~4 chunks each): the consuming
                # matmuls wait per-call, so TensorE starts ~6us earlier per
                # block and the end-of-hop backlog shrinks to one call
                for c0 in range(0, nch, 2):
                    c1 = min(c0 + 2, nch)
                    nc.gpsimd.dma_gather(
                        out_ap=g[:, c0:c1, :],
                        in_ap=src_ap,
                        idxs_ap=idx_t[:, (off + c0) * 8:(off + c1) * 8],
                        num_idxs=(c1 - c0) * 128,
                        num_idxs_reg=(c1 - c0) * 128,
                        elem_size=D,
                        single_packet=False,
                    )
                acc = psum.tile([128, D], dt.float32, tag="agg", name="acc")
                for c in range(nch):
                    nc.tensor.matmul(acc[:], s_t[:, off + c, :], g[:, c, :],
                                     start=(c == 0), stop=(c == nch - 1))
                hb = evict.tile([128, D], dt.bfloat16, tag="hb", name="hb")
                nc.vector.tensor_copy(hb[:], acc[:])
                tps = ptp.tile([128, FI, 128], dt.bfloat16, tag="tps", name="tps")
                for fi in range(FI):
                    nc.tensor.transpose(
                        tps[:, fi, :], hb[:, fi * 128:(fi + 1) * 128], ident_t[:])
                nc.vector.tensor_copy(
                    ht[k][:, :, bi * 128:bi * 128 + 128], tps[:])
                return hb

            linear(0)

            # hop 1: gather from the full local h0 replica; shard writes go
            # on the (otherwise idle) sync ring so the AllGather input is
            # not queued behind linear(1) output writes
            last_write = None
            for bi in range(BLKS):
                hb = hop_block(bi, 1, idx1_t, h0_full[:])
                last_write = nc.sync.dma_start(
                    h1_shard[bi * 128:(bi + 1) * 128, :], hb[:])

            # single AllGather of the local h1 shard (1.28MB in, 10.2MB out)
            nc.gpsimd.collective_compute(
                "AllGather",
                mybir.AluOpType.bypass,
                replica_groups=[list(range(N_CORES))],
                ins=[h1_shard[:SHARD, :]],
                outs=[h1_blocked[:]],
            )

            # linear(1): with realistic gather costs the scheduler places
            # these units into TensorE idle gaps (keeping the PE warm)
            # without delaying hop-1's tail chain
            linear(1)

            # hop 2: gather from the all-gathered blocked h1 copy. Blocks
            # 8,9 first so linear(2)'s last (smallest) node group finishes
            # early; each linear(2) group is emitted as soon as the ht[2]
            # columns it reads are complete, trimming the kernel tail.
            def linear_grp(k, gi):
                gsz = NGRPS[gi]
                goff = sum(NGRPS[:gi])
                for ft in range(FO):
                    pw = psw.tile([128, gsz], dt.float32, tag="pw", name="pw")
                    for fi in range(FI):
                        nc.tensor.matmul(
                            pw[:], w_t[:, k, fi, ft * 128:(ft + 1) * 128],
                            ht[k][:, fi, goff:goff + gsz],
                            start=(fi == 0), stop=(fi == FI - 1))
                    ob = lin.tile([128, gsz], dt.float32, tag="ob", name="ob")
                    nc.scalar.activation(
                        out=ob[:], in_=pw[:],
                        func=mybir.ActivationFunctionType.Relu,
                        bias=b_t[:, k, ft:ft + 1])
                    nc.scalar.dma_start(
                        out_t[k * D + ft * 128:k * D + (ft + 1) * 128,
                              goff:goff + gsz], ob[:])

            for step, bi in enumerate([8, 9] + list(range(8))):
                hop_block(bi, 2, idx1_t, h1_blocked[:])
                if bi == 9:
                    linear_grp(2, 2)
                elif bi == 3:
                    linear_grp(2, 0)
                elif bi == 7:
                    linear_grp(2, 1)

    nc.compile()
    _assign_gather_queues(nc)
    _split_excess_waits(nc)
    return nc


def kernel(features, D_norm, edge_w, W, b, src, dst, _timing=None):
    features = np.asarray(features, np.float32)
    D_norm = np.asarray(D_norm, np.float32)
    edge_w = np.asarray(edge_w, np.float32)
    W = np.asarray(W, np.float32)
    b = np.asarray(b, np.float32)
    src = np.asarray(src, np.int32)
    dst = np.asarray(dst, np.int32)

    pre = _preprocess(edge_w, src, dst, D_norm[:, 0])
    nchk = np.max(np.stack([p[0] for p in pre]), axis=0)
    ncht = int(nchk.sum())
    nc = _build_program(nchk, ncht)

    h0_bf = features.astype(BF16)
    w_pack = np.zeros((128, ORDER + 1, FI, D), np.float32)
    for fi in range(FI):
        w_pack[:, :, fi, :] = W[:, fi * 128:(fi + 1) * 128, :].transpose(1, 0, 2)
    b_pack = np.zeros((128, ORDER + 1, FO), np.float32)
    for ft in range(FO):
        b_pack[:, :, ft] = b[:, ft * 128:(ft + 1) * 128].T

    def pad_idx(nchk_loc, flat, nchk_pad, ncht_pad):
        """Relayout a flat per-chunk idx/S table into the shared padded
        shape, then wrap into the [128, ncht*8] gather-index layout."""
        off_l = np.concatenate([[0], np.cumsum(nchk_loc)])
        off_p = np.concatenate([[0], np.cumsum(nchk_pad)])
        out = np.zeros(ncht_pad * 128, flat.dtype)
        for bi in range(len(nchk_loc)):
            nl = int(nchk_loc[bi])
            out[int(off_p[bi]) * 128:(int(off_p[bi]) + nl) * 128] = \
                flat[int(off_l[bi]) * 128:(int(off_l[bi]) + nl) * 128]
        return np.tile(out.reshape(-1, 16).T, (8, 1))

    def pad_s(nchk_loc, s_loc, nchk_pad, ncht_pad):
        off_l = np.concatenate([[0], np.cumsum(nchk_loc)])
        off_p = np.concatenate([[0], np.cumsum(nchk_pad)])
        out = np.zeros((128, ncht_pad, 128), BF16)
        for bi in range(len(nchk_loc)):
            nl = int(nchk_loc[bi])
            out[:, int(off_p[bi]):int(off_p[bi]) + nl, :] = \
                s_loc[:, int(off_l[bi]):int(off_l[bi]) + nl, :]
        return out

    ident = np.eye(128, dtype=BF16)
    in_maps = []
    for i in range(N_CORES):
        sh = slice(i * SHARD, (i + 1) * SHARD)
        h0t = features[sh].reshape(SHARD, FI, 128).transpose(2, 1, 0)
        nchk_loc, idx1_f, s_loc = pre[i]
        in_maps.append({
            "h0_full": h0_bf,
            "h0t_shard": h0t.astype(BF16).copy(),
            "idx1": pad_idx(nchk_loc, idx1_f, nchk, ncht),
            "s": pad_s(nchk_loc, s_loc, nchk, ncht),
            "ident": ident,
            "w": w_pack.astype(BF16),
            "bias": b_pack,
        })

    res = run_bass_kernel_spmd(
        nc, in_maps, list(range(N_CORES)),
        trace=bool(_timing is not None))
    if _timing is not None:
        _timing["exec_time_ns"] = res.exec_time_ns

    parts = [np.asarray(res.results[i]["out_t"]) for i in range(N_CORES)]
    out = np.concatenate(parts, axis=1).T          # [N, 3*D]
    return np.ascontiguousarray(out, dtype=np.float32)


# revision 34
# speedup vs baseline: 1.1578x; 1.1578x over previous
"""GCN layer (2-hop SpMM + per-hop Linear/ReLU) on 8 Trainium2 NeuronCores.

Strategy (dst-sharded pull, both hops; single AllGather between them):
  - Nodes sharded 1250/core; each core owns the edges pointing at its
    shard. Both hops use the SAME edge set, so one host-built scatter
    table S serves both: per dst block, edges are grouped by UNIQUE src
    (dedup: each distinct src row is gathered once per block) into
    128-lane chunks; S[lane, chunk, m] = sum of edge weights from that
    src to local dst m.
  - The SWDGE descriptor-generation rate on GpSimd Q7 cores 0/1 (~4.7ns
    per gathered row, 2-way overlap max) is the kernel's critical
    resource; ncfw collectives also occupy the GpSimd queue for their
    full duration. The layout therefore minimizes that serial chain:
    hop-1 gathers (local replicated h0) -> one AllGather of the 1.25MB
    h1 shard -> hop-2 gathers (local h1_blocked copy).
  - Per hop/block: dma_gather pulls rows (bf16) into SBUF G tiles;
    TensorE computes psum += S.T @ G per chunk (scatter-add; D_norm is
    folded into S host-side, valid for both hops since they share dst
    sharding); VectorE evicts psum to bf16. The feat-major copy for the
    linear layers is built with TensorE transpose-mode matmuls + DVE
    copies — NOT dma_start_transpose: Tile serializes DMA-transposes
    against SWDGE gathers (HW-deadlock workaround), which was measured
    to cap the gather pipeline at ~2 blocks in flight.
  - Linear stage runs feat-major: outT[fo, n] = relu(W.T @ hT + b), bias
    and relu fused in one ScalarE activation. linear(0) warms TensorE at
    the start; linear(1) is interleaved into early hop-2 blocks.
  - HWDGE traffic is split across both rings: nc.sync (SP) carries the
    urgent const loads + transposes, nc.scalar (ACT) carries the hop-2
    index table, h1 shard write-out and all output writes.
"""

import sys

sys.path.insert(0, "/opt/trn_rl_repo")

import numpy as np
import ml_dtypes

import concourse.bass as bass
import concourse.bacc as bacc
import concourse.mybir as mybir
import concourse.tile as tile
from concourse import library_config
from concourse.bass_utils import run_bass_kernel_spmd
from concourse.tile_rust import add_dep_helper

N_NODES = 10000
N_EDGES = 160000
D = 512
ORDER = 2
N_CORES = 8
SHARD = N_NODES // N_CORES          # 1250
BLKS = (SHARD + 127) // 128         # 10 dst blocks per core
BLK_SZ = [min(128, SHARD - b * 128) for b in range(BLKS)]  # [128]*9 + [98]
FI = D // 128                       # 4 feat-in chunks
FO = D // 128                       # 4 feat-out tiles
NGRPS = [512, 512, SHARD - 1024]    # node groups for linear stage
PAD_SHARD = BLKS * 128              # 1280 rows per core in blocked layout
BF16 = ml_dtypes.bfloat16


def _split_excess_waits(nc, max_waits=1):
    """This walrus build rejects >1 sync wait per instruction (and any on a
    Drain). Hoist excess SyncWaits onto InstNoOp carriers inserted just
    before, on the same engine — waits execute in program order, so
    semantics are preserved."""
    for fn in nc.m.functions:
        for bb in fn.blocks:
            new = []
            changed = False
            for inst in bb.instructions:
                si = inst.sync_info
                cap = 0 if isinstance(inst, mybir.InstDrain) else max_waits
                if si is not None and len(si.on_wait) > cap:
                    waits = list(si.on_wait)
                    excess = waits[:-cap] if cap else waits
                    keep = waits[-cap:] if cap else []
                    for g in range(0, len(excess), max_waits):
                        nop = mybir.InstNoOp(name=f"{inst.name}-ws{g}", ins=[], outs=[])
                        nop.engine = inst.engine
                        nop.sync_info = mybir.SyncInfo(
                            on_wait=excess[g:g + max_waits], on_update=[])
                        new.append(nop)
                    si.on_wait = keep
                    changed = True
                new.append(inst)
            if changed:
                bb.instructions = new


def _assign_gather_queues(nc):
    """Tile locks each DMASW sem lane to SWDGE queue lane%4; route every
    gather through the queue matching its (scheduler-assigned) sem lane so
    the 4 SWDGE queues actually run in parallel."""
    for fn in nc.m.functions:
        for bb in fn.blocks:
            for inst in bb.instructions:
                if isinstance(inst, mybir.InstDMAGatherAnt):
                    si = inst.sync_info
                    if not si:
                        continue
                    for u in si.on_update:
                        nm = getattr(u, "ant_name", "") or ""
                        if nm.startswith("DMASW"):
                            lane = int(nm[5:].split("_")[0])
                            inst.queue_num = lane % 4
                            break


def _blocked_row(s):
    """Row of global node s in the AllGather output layout
    h1_blocked[core * 1280 + local_row]."""
    return (s // SHARD) * PAD_SHARD + (s % SHARD)


def _preprocess(edge_w, src, dst, d_flat):
    """Host-side: shard edges by dst owner; per dst block, dedup by src
    into 128-lane chunks shared by both hops; D_norm[dst] is folded into
    the scatter weights. Returns per-core (nchk [BLKS], idx1, idx2, S)."""
    core_of = dst // SHARD
    per_core = []
    for i in range(N_CORES):
        sel = np.nonzero(core_of == i)[0]
        dl = dst[sel] - i * SHARD
        blk = dl // 128
        m = dl % 128
        s = src[sel]
        w = edge_w[sel] * d_flat[dst[sel]]
        nchk = np.zeros(BLKS, np.int64)
        cells = []
        for bi in range(BLKS):
            bm = blk == bi
            u, inv = np.unique(s[bm], return_inverse=True)
            nchk[bi] = max((len(u) + 127) // 128, 1)
            cells.append((u, inv, m[bm], w[bm]))
        ncht = int(nchk.sum())
        boff = np.concatenate([[0], np.cumsum(nchk)])
        idx1_f = np.zeros(ncht * 128, np.int16)
        s_t = np.zeros((128, ncht, 128), np.float32)
        for bi in range(BLKS):
            u, inv, mm, ww = cells[bi]
            lo = int(boff[bi])
            idx1_f[lo * 128: lo * 128 + len(u)] = u.astype(np.int16)
            np.add.at(s_t, (inv % 128, lo + inv // 128, mm), ww)
        per_core.append((nchk, idx1_f, s_t.astype(BF16)))
    return per_core


def _build_program(nchk, ncht):
    # The stock cost model prices SWDGE descriptor generation at 0.34ns
    # (calibrated on plain dma_start); dma_gather's data-dependent Q7 loop
    # measures ~8.7ns/index on HW. With the optimistic value the Tile list
    # scheduler interleaves linear-stage matmuls ahead of hop chains it
    # believes will stall, delaying the AllGather. Patch the constant so
    # schedule-time gather latencies match reality (no effect on the
    # generated instructions themselves, only their ordering).
    import concourse.hw_specs as hw_specs
    hw_specs.TRN2Spec.SWDGE_NS_PER_DESCRIPTOR = 7.0

    nc = bacc.Bacc("TRN2", num_swdge_queues=4)
    dt = mybir.dt

    h0_full = nc.declare_dram_parameter("h0_full", [N_NODES, D], dt.bfloat16, isOutput=False)
    h0t_shard = nc.declare_dram_parameter("h0t_shard", [128, FI, SHARD], dt.bfloat16, isOutput=False)
    idx1_in = nc.declare_dram_parameter("idx1", [128, ncht * 8], dt.int16, isOutput=False)
    s_in = nc.declare_dram_parameter("s", [128, ncht, 128], dt.bfloat16, isOutput=False)
    ident_in = nc.declare_dram_parameter("ident", [128, 128], dt.bfloat16, isOutput=False)
    w_in = nc.declare_dram_parameter("w", [128, ORDER + 1, FI, D], dt.bfloat16, isOutput=False)
    b_in = nc.declare_dram_parameter("bias", [128, ORDER + 1, FO], dt.float32, isOutput=False)
    out_t = nc.declare_dram_parameter("out_t", [(ORDER + 1) * D, SHARD], dt.float32, isOutput=True)

    # the AllGather of exact [SHARD]-row shards reproduces global node
    # order, so hop-2 gathers reuse idx1 (h1_blocked[src] = h1[src])
    h1_shard = nc.dram_tensor("h1_shard", [PAD_SHARD, D], dt.bfloat16)
    h1_blocked = nc.dram_tensor("h1_blocked", [N_NODES, D], dt.bfloat16,
                                addr_space="Shared")

    boff = np.concatenate([[0], np.cumsum(nchk)])

    with tile.TileContext(nc) as tc:
        nc.gpsimd.load_library(library_config.mlp)
        with (
            tc.tile_pool(name="const", bufs=1) as const,
            tc.tile_pool(name="gbuf", bufs=6) as gbuf,
            tc.tile_pool(name="evict", bufs=3) as evict,
            tc.tile_pool(name="lin", bufs=3) as lin,
            tc.tile_pool(name="psum", bufs=3, space=bass.MemorySpace.PSUM) as psum,
            tc.tile_pool(name="ptp", bufs=2, space=bass.MemorySpace.PSUM) as ptp,
            tc.tile_pool(name="psw", bufs=3, space=bass.MemorySpace.PSUM) as psw,
        ):
            # urgent first (sync ring): gather idx, weights, ht0 feed the
            # first gathers + linear(0); the big S table follows them.
            idx1_t = const.tile([128, ncht * 8], dt.int16)
            nc.sync.dma_start(idx1_t[:], idx1_in[:])
            ident_t = const.tile([128, 128], dt.bfloat16)
            nc.sync.dma_start(ident_t[:], ident_in[:])
            w_t = const.tile([128, ORDER + 1, FI, D], dt.bfloat16)
            nc.sync.dma_start(w_t[:], w_in[:])
            b_t = const.tile([128, ORDER + 1, FO], dt.float32)
            nc.sync.dma_start(b_t[:], b_in[:])
            ht = [const.tile([128, FI, PAD_SHARD], dt.bfloat16, tag=f"ht{k}",
                             name=f"ht{k}")
                  for k in range(ORDER + 1)]
            nc.sync.dma_start(ht[0][:, :, :SHARD], h0t_shard[:])
            # big table on the ACT ring so its load doesn't alias the
            # gathers' DMAHW wait lanes on the sync ring
            s_t = const.tile([128, ncht, 128], dt.bfloat16)
            nc.scalar.dma_start(s_t[:], s_in[:])

            def linear(k, fts=None, after=None):
                for ft in (range(FO) if fts is None else fts):
                    for gi, gsz in enumerate(NGRPS):
                        goff = sum(NGRPS[:gi])
                        pw = psw.tile([128, gsz], dt.float32, tag="pw", name="pw")
                        for fi in range(FI):
                            mm = nc.tensor.matmul(
                                pw[:], w_t[:, k, fi, ft * 128:(ft + 1) * 128],
                                ht[k][:, fi, goff:goff + gsz],
                                start=(fi == 0), stop=(fi == FI - 1))
                            if after is not None and fi == 0:
                                add_dep_helper(mm.ins, after.ins, sync=True,
                                               reason="linear after hop chain")
                        ob = lin.tile([128, gsz], dt.float32, tag="ob", name="ob")
                        nc.scalar.activation(
                            out=ob[:], in_=pw[:],
                            func=mybir.ActivationFunctionType.Relu,
                            bias=b_t[:, k, ft:ft + 1])
                        nc.scalar.dma_start(
                            out_t[k * D + ft * 128:k * D + (ft + 1) * 128,
                                  goff:goff + gsz], ob[:])

            def hop_block(bi, k, idx_t, src_ap):
                """One dst block of one SpMM hop (shared chunk table)."""
                nch = int(nchk[bi])
                off = int(boff[bi])
                g = gbuf.tile([128, nch, D], dt.bfloat16, tag="g", name="g")
                # fine-grained gather calls (~4 chunks each): the consuming
                # matmuls wait per-call, so TensorE starts ~6us earlier per
                # block and the end-of-hop backlog shrinks to one call
                # (2-chunk calls measured WORSE: DMASW lane-cap pressure)
                for c0 in range(0, nch, 4):
                    c1 = min(c0 + 4, nch)
                    nc.gpsimd.dma_gather(
                        out_ap=g[:, c0:c1, :],
                        in_ap=src_ap,
                        idxs_ap=idx_t[:, (off + c0) * 8:(off + c1) * 8],
                        num_idxs=(c1 - c0) * 128,
                        num_idxs_reg=(c1 - c0) * 128,
                        elem_size=D,
                        single_packet=False,
                    )
                acc = psum.tile([128, D], dt.float32, tag="agg", name="acc")
                for c in range(nch):
                    nc.tensor.matmul(acc[:], s_t[:, off + c, :], g[:, c, :],
                                     start=(c == 0), stop=(c == nch - 1))
                hb = evict.tile([128, D], dt.bfloat16, tag="hb", name="hb")
                nc.vector.tensor_copy(hb[:], acc[:])
                tps = ptp.tile([128, FI, 128], dt.bfloat16, tag="tps", name="tps")
                for fi in range(FI):
                    nc.tensor.transpose(
                        tps[:, fi, :], hb[:, fi * 128:(fi + 1) * 128], ident_t[:])
                nc.vector.tensor_copy(
                    ht[k][:, :, bi * 128:bi * 128 + 128], tps[:])
                return hb

            linear(0)

            # hop 1: gather from the full local h0 replica; shard writes go
            # on the (otherwise idle) sync ring so the AllGather input is
            # not queued behind linear(1) output writes
            last_write = None
            for bi in range(BLKS):
                hb = hop_block(bi, 1, idx1_t, h0_full[:])
                last_write = nc.sync.dma_start(
                    h1_shard[bi * 128:(bi + 1) * 128, :], hb[:])

            # single AllGather of the local h1 shard (1.28MB in, 10.2MB out)
            nc.gpsimd.collective_compute(
                "AllGather",
                mybir.AluOpType.bypass,
                replica_groups=[list(range(N_CORES))],
                ins=[h1_shard[:SHARD, :]],
                outs=[h1_blocked[:]],
            )

            # linear(1): with realistic gather costs the scheduler places
            # these units into TensorE idle gaps (keeping the PE warm)
            # without delaying hop-1's tail chain
            linear(1)

            # hop 2: gather from the all-gathered blocked h1 copy. Blocks
            # 8,9 first so linear(2)'s last (smallest) node group finishes
            # early; each linear(2) group is emitted as soon as the ht[2]
            # columns it reads are complete, trimming the kernel tail.
            def linear_grp(k, gi):
                gsz = NGRPS[gi]
                goff = sum(NGRPS[:gi])
                for ft in range(FO):
                    pw = psw.tile([128, gsz], dt.float32, tag="pw", name="pw")
                    for fi in range(FI):
                        nc.tensor.matmul(
                            pw[:], w_t[:, k, fi, ft * 128:(ft + 1) * 128],
                            ht[k][:, fi, goff:goff + gsz],
                            start=(fi == 0), stop=(fi == FI - 1))
                    ob = lin.tile([128, gsz], dt.float32, tag="ob", name="ob")
                    nc.scalar.activation(
                        out=ob[:], in_=pw[:],
                        func=mybir.ActivationFunctionType.Relu,
                        bias=b_t[:, k, ft:ft + 1])
                    nc.scalar.dma_start(
                        out_t[k * D + ft * 128:k * D + (ft + 1) * 128,
                              goff:goff + gsz], ob[:])

            for step, bi in enumerate([8, 9] + list(range(8))):
                hop_block(bi, 2, idx1_t, h1_blocked[:])
                if bi == 9:
                    linear_grp(2, 2)
                elif bi == 3:
                    linear_grp(2, 0)
                elif bi == 7:
                    linear_grp(2, 1)

    nc.compile()
    _assign_gather_queues(nc)
    _split_excess_waits(nc)
    return nc


def kernel(features, D_norm, edge_w, W, b, src, dst, _timing=None):
    features = np.asarray(features, np.float32)
    D_norm = np.asarray(D_norm, np.float32)
    edge_w = np.asarray(edge_w, np.float32)
    W = np.asarray(W, np.float32)
    b = np.asarray(b, np.float32)
    src = np.asarray(src, np.int32)
    dst = np.asarray(dst, np.int32)

    pre = _preprocess(edge_w, src, dst, D_norm[:, 0])
    nchk = np.max(np.stack([p[0] for p in pre]), axis=0)
    ncht = int(nchk.sum())
    nc = _build_program(nchk, ncht)

    h0_bf = features.astype(BF16)
    w_pack = np.zeros((128, ORDER + 1, FI, D), np.float32)
    for fi in range(FI):
        w_pack[:, :, fi, :] = W[:, fi * 128:(fi + 1) * 128, :].transpose(1, 0, 2)
    b_pack = np.zeros((128, ORDER + 1, FO), np.float32)
    for ft in range(FO):
        b_pack[:, :, ft] = b[:, ft * 128:(ft + 1) * 128].T

    def pad_idx(nchk_loc, flat, nchk_pad, ncht_pad):
        """Relayout a flat per-chunk idx/S table into the shared padded
        shape, then wrap into the [128, ncht*8] gather-index layout."""
        off_l = np.concatenate([[0], np.cumsum(nchk_loc)])
        off_p = np.concatenate([[0], np.cumsum(nchk_pad)])
        out = np.zeros(ncht_pad * 128, flat.dtype)
        for bi in range(len(nchk_loc)):
            nl = int(nchk_loc[bi])
            out[int(off_p[bi]) * 128:(int(off_p[bi]) + nl) * 128] = \
                flat[int(off_l[bi]) * 128:(int(off_l[bi]) + nl) * 128]
        return np.tile(out.reshape(-1, 16).T, (8, 1))

    def pad_s(nchk_loc, s_loc, nchk_pad, ncht_pad):
        off_l = np.concatenate([[0], np.cumsum(nchk_loc)])
        off_p = np.concatenate([[0], np.cumsum(nchk_pad)])
        out = np.zeros((128, ncht_pad, 128), BF16)
        for bi in range(len(nchk_loc)):
            nl = int(nchk_loc[bi])
            out[:, int(off_p[bi]):int(off_p[bi]) + nl, :] = \
                s_loc[:, int(off_l[bi]):int(off_l[bi]) + nl, :]
        return out

    ident = np.eye(128, dtype=BF16)
    in_maps = []
    for i in range(N_CORES):
        sh = slice(i * SHARD, (i + 1) * SHARD)
        h0t = features[sh].reshape(SHARD, FI, 128).transpose(2, 1, 0)
        nchk_loc, idx1_f, s_loc = pre[i]
        in_maps.append({
            "h0_full": h0_bf,
            "h0t_shard": h0t.astype(BF16).copy(),
            "idx1": pad_idx(nchk_loc, idx1_f, nchk, ncht),
            "s": pad_s(nchk_loc, s_loc, nchk, ncht),
            "ident": ident,
            "w": w_pack.astype(BF16),
            "bias": b_pack,
        })

    res = run_bass_kernel_spmd(
        nc, in_maps, list(range(N_CORES)),
        trace=bool(_timing is not None))
    if _timing is not None:
        _timing["exec_time_ns"] = res.exec_time_ns

    parts = [np.asarray(res.results[i]["out_t"]) for i in range(N_CORES)]
    out = np.concatenate(parts, axis=1).T          # [N, 3*D]
    return np.ascontiguousarray(out, dtype=np.float32)
